# revision 44
# baseline (speedup 1.0000x reference)
"""Causal self-attention on 8 trn2 NeuronCores (bf16 datapath, fp8-ready).

Problem: B=2, T=2048, C=1024, 16 heads of 64. Sharding: core = 4*b + g
(b = batch, g = head-group of 4 heads). Each core computes QKV projection
for its 4 heads, causal attention, and a partial c_proj (its 256 rows of
w_proj). Host sums the 4 partials per batch (the "all-reduce") + b_proj.

The PE is the binding engine (~100us of matmul at bf16; Act exp is
~76us; DVE/Pool less), so the schedule exists to keep the PE queue fed:
every deferrable matmul (pair-1 q/k projection, V tiles, c_proj) is
emitted as filler inside the exp-gated attention phases.

Layouts are slot-paired ([p, 2, *]) so each matmul class can switch
between one fp8 DoubleRow instruction and two plain bf16 matmuls via
the knobs below (numerics currently demand bf16 everywhere):

  xt[s]   (128, 2, 2048)  x^T, c-tile pair s (c = 256s + 128j + p)
  wqk[s]  (128, 2, 512)   chunks m: q-pair0, q-pair1, k-pair0, k-pair1
  qk[m]   (128, 2048)     q^T/k^T per head-pair chunk (bf16: S matmul
                          operand base partitions are limited to
                          {0,32,64}, ruling out a 4x32 fp8-DR S layout)
  v_all   (128, 16, 4, 65) V'=[V|1] natural; slot pairs = adjacent tk
  pt2     (128, 2, 1024)  P^T pairs: slot = tk-tile parity
  yt      (128, 2, 2048)  y^T: slot = head pair
  wp2     (128, 2, 1024)  w_proj rows: slot = head pair

Attention: head-serial (pair, half, h2) phases, software-pipelined:
S(t+1) and fillers are emitted before the AV burst that parks on
exp(t); exp applies scale=1/8 (weights are kept unscaled) and bias=-2
(rescales P by e^-2, cancels in the softmax ratio; keeps exp inside
fp8 range if pt ever goes fp8). The diagonal block is masked by a DVE
multiply with the inclusive-upper-tri mask. AV runs flipped (stationary
P^T slice, moving V' -> out [tq, 65]); the denominator lands as column
65, normalization is reciprocal + tensor_scalar on DVE, and a PE
transpose (identity moving operand) rebuilds y^T.

c_proj is split in time: the pair-0 half (yt slot 0) is computed into
SBUF partials during phases (0,1,*) and (1,0,*) where the PE would
otherwise idle; the pair-1 half + merge-add + out DMA ride (1,1,*) as
the pair-1 yt columns complete. Act does exp only; Pool (gpsimd) takes
bias adds, tp->yt copies and half of the partial/merge traffic.
"""

import numpy as np
import ml_dtypes

import concourse.tile as tile
from concourse import bacc, mybir
from concourse.bass_utils import run_bass_kernel_spmd

B, T, C = 2, 2048, 1024
HS = 64
NCORES = 8
NHL = 4            # heads per core
TCH = 512
NT = T // 128      # 16 tk tiles
F32 = mybir.dt.float32
BF16 = mybir.dt.bfloat16
F8 = mybir.dt.float8e4
E4 = ml_dtypes.float8_e4m3

# dtype knobs: fp8+DoubleRow per stage (False = bf16, two plain matmuls)
XW8 = False  # x / wqk / wv: QKV projection
PV8 = False  # P / V: AV matmul
YP8 = False  # y^T / wp: c_proj

DR = mybir.MatmulPerfMode.DoubleRow


def build_program():
    nc = bacc.Bacc("TRN2", target_bir_lowering=False, debug=False)

    xdt = F8 if XW8 else BF16
    vdt = F8 if PV8 else BF16
    ydt = F8 if YP8 else BF16

    x4_d = nc.dram_tensor("x4", [128, 4, 2, T], xdt,
                          kind="ExternalInput").ap()
    wqk_d = nc.dram_tensor("wqk", [128, 4, 2, 512], xdt,
                           kind="ExternalInput").ap()
    wv_d = nc.dram_tensor("wv", [128, 4, 2, 256], xdt,
                          kind="ExternalInput").ap()
    wp_d = nc.dram_tensor("wp", [128, 2, 1024], ydt,
                          kind="ExternalInput").ap()
    bqk_d = nc.dram_tensor("bqk", [128, 4], F32, kind="ExternalInput").ap()
    bvb_d = nc.dram_tensor("bvb", [128, 320], BF16,
                           kind="ExternalInput").ap()
    msk_d = nc.dram_tensor("msk", [128, 128], BF16,
                           kind="ExternalInput").ap()
    idn_d = nc.dram_tensor("idn", [128, 128], BF16,
                           kind="ExternalInput").ap()
    out_d = nc.dram_tensor("out", [T, C], BF16, kind="ExternalOutput").ap()

    with tile.TileContext(nc) as tc:
        _kernel(tc, out_d, x4_d, wqk_d, wv_d, wp_d, bqk_d, bvb_d, msk_d,
                idn_d, xdt, vdt, ydt)
    nc.compile()
    return nc


def _kernel(tc, out_d, x4_d, wqk_d, wv_d, wp_d, bqk_d, bvb_d, msk_d,
            idn_d, xdt, vdt, ydt):
    nc = tc.nc
    AF = mybir.ActivationFunctionType

    def mm2(out, lhsT, rhs, start, stop, f8):
        # slot-paired matmul: lhsT/rhs [p, 2, *]. One DoubleRow fp8
        # instruction, or two plain matmuls over the slots.
        if f8:
            nc.tensor.matmul(out, lhsT, rhs, start=start, stop=stop,
                             perf_mode=DR)
        else:
            nc.tensor.matmul(out, lhsT[:, 0], rhs[:, 0], start=start,
                             stop=False)
            nc.tensor.matmul(out, lhsT[:, 1], rhs[:, 1], start=False,
                             stop=stop)

    with (
        tc.tile_pool(name="persist", bufs=1) as pers,
        tc.tile_pool(name="ps", bufs=2, space="PSUM") as ps,
        tc.tile_pool(name="po", bufs=4, space="PSUM") as po,
    ):
        # zero operands for PE-warmup matmuls; memset first so the ramp
        # warmups start before any DMA lands
        zs = pers.tile([64, 128], BF16, tag="zs")
        nc.vector.memset(zs[:], 0)
        zs2 = pers.tile([64, 512], BF16, tag="zs2")
        nc.vector.memset(zs2[:], 0)
        neg2 = pers.tile([128, 1], F32, tag="neg2")
        nc.vector.memset(neg2[:], -2.0)

        xp = tc.alloc_tile_pool(name="xp", bufs=1)
        xt = []
        for s in range(4):
            t_ = xp.tile([128, 2, T], xdt, tag=f"xt{s}", name=f"xt{s}")
            # two sub-DMAs per c-tile pair so consumption paces the stream
            nc.sync.dma_start(out=t_[:, 0, :], in_=x4_d[:, s, 0])
            nc.sync.dma_start(out=t_[:, 1, :], in_=x4_d[:, s, 1])
            xt.append(t_)
        wqk = []
        for s in range(4):
            w_ = pers.tile([128, 2, 512], xdt, tag=f"wqk{s}",
                           name=f"wqk{s}")
            nc.scalar.dma_start(out=w_, in_=wqk_d[:, s])
            wqk.append(w_)
        bqk = pers.tile([128, 4], F32, tag="bqk")
        nc.scalar.dma_start(out=bqk, in_=bqk_d)
        msk = pers.tile([128, 128], BF16, tag="msk")
        nc.scalar.dma_start(out=msk, in_=msk_d)
        wv = []
        for s in range(4):
            t_ = pers.tile([128, 2, 256], xdt, tag=f"wv{s}", name=f"wv{s}")
            nc.sync.dma_start(out=t_, in_=wv_d[:, s])
            wv.append(t_)
        bvb = pers.tile([128, 320], BF16, tag="bvb")
        nc.sync.dma_start(out=bvb, in_=bvb_d)
        idn = pers.tile([128, 128], BF16, tag="idn")
        nc.scalar.dma_start(out=idn, in_=idn_d)
        wp2 = pers.tile([128, 2, 1024], ydt, tag="wp2")
        nc.scalar.dma_start(out=wp2, in_=wp_d)

        # q^T / k^T chunks: m 0/1 -> q head-pairs, 2/3 -> k head-pairs
        qk = [pers.tile([128, T], BF16, tag=f"qk{m}", name=f"qk{m}")
              for m in range(4)]
        # V' = [V | 1] per (tk-tile, head); ones col from bvb[:, 256:320]
        v_all = pers.tile([128, NT, NHL, HS + 1], vdt, tag="v_all",
                          name="v_all")
        nc.vector.tensor_copy(
            out=v_all[:, :, :, HS],
            in_=bvb[:, 256:320].rearrange("p (a b) -> p a b", a=NT),
        )
        # y^T, slot = head pair
        yt = pers.tile([128, 2, T], ydt, tag="yt", name="yt")

        def warm(pst, n):
            for i in range(n):
                nc.tensor.matmul(
                    pst[0:128, 0:512], zs[:], zs2[:],
                    start=(i == 0), stop=False, skip_group_check=True,
                )

        # ---- QKV projection group (m-chunk, tq i-chunk) ----
        def qk_add(m, i, pg):
            eng = nc.vector if (m + i) % 2 == 0 else nc.gpsimd
            eng.tensor_scalar_add(
                out=qk[m][:, TCH * i:TCH * (i + 1)],
                in0=pg[:],
                scalar1=bqk[:, m:m + 1],
            )

        def qk_sub(m, i):
            pg = po.tile([128, TCH], F32, tag="po", name=f"pq{m}_{i}")
            for s in range(4):
                mm2(pg[:], wqk[s][:, :, 128 * m:128 * (m + 1)],
                    xt[s][:, :, TCH * i:TCH * (i + 1)], s == 0, s == 3,
                    XW8)
            qk_add(m, i, pg)

        def qk_upfront():
            # pair-0 q/k (m=0,2) x all i: 8 concurrent groups (4 po +
            # 2x2 ps halves), s-major so the PE paces the x DMA stream
            pgs = []
            for i in range(2):
                for m in (0, 2):
                    pgs.append((m, i, po.tile([128, TCH], F32, tag="po",
                                              name=f"pg{m}_{i}")))
            wide = [ps.tile([128, 1024], F32, tag="st", name=f"pw{i}")
                    for i in range(2)]
            for i in range(2, 4):
                for m in (0, 2):
                    pgs.append((m, i, wide[i - 2][:, TCH * (m // 2):
                                                  TCH * (m // 2 + 1)]))
            warm(pgs[0][2], 3)
            for s in range(4):
                for m, i, pg in pgs:
                    mm2(pg[:], wqk[s][:, :, 128 * m:128 * (m + 1)],
                        xt[s][:, :, TCH * i:TCH * (i + 1)], s == 0,
                        s == 3, XW8)
            for m, i, pg in pgs:
                qk_add(m, i, pg)

        # ---- V tile t (natural layout, +bias) ----
        def v_tile(t):
            vp = po.tile([128, 256], F32, tag="po", name=f"vp{t}")
            for s in range(4):
                mm2(vp[:], xt[s][:, :, 128 * t:128 * (t + 1)], wv[s][:],
                    s == 0, s == 3, XW8)
            nc.gpsimd.tensor_add(
                out=v_all[:, t, :, 0:HS],
                in0=vp[:].rearrange("p (h d) -> p h d", h=NHL),
                in1=bvb[:, 0:256].rearrange("p (h d) -> p h d", h=NHL),
            )

        # ---- attention-side SBUF pools ----
        ptp = tc.alloc_tile_pool(name="pt", bufs=14)
        lrp = tc.alloc_tile_pool(name="lrec", bufs=8)
        ynp = tc.alloc_tile_pool(name="ynp", bufs=40)
        pp0p = tc.alloc_tile_pool(name="pp0", bufs=16)
        ostp = tc.alloc_tile_pool(name="ost", bufs=4)
        yn = [[None] * NT for _ in range(2)]
        pp0 = [None] * NT
        pending_tp = []

        def drain_j(pair, h2, j, av):
            if h2 == 0:
                yn[pair][j] = ynp.tile([128, 2, HS], BF16, tag="yn",
                                       name=f"yn{pair}{j}")
            lr = lrp.tile([128, 1], F32, tag="lr", name=f"lr{pair}{h2}{j}")
            nc.vector.reciprocal(out=lr[:], in_=av[:, HS:HS + 1])
            nc.vector.tensor_scalar_mul(
                out=yn[pair][j][:, h2, :],
                in0=av[:, 0:HS],
                scalar1=lr[:],
            )
            if h2 == 1:
                pending_tp.append((pair, j))

        def flush_tp(force=True):
            if not force and len(pending_tp) < 2:
                return
            while pending_tp:
                pair, j = pending_tp.pop(0)
                tp = po.tile([128, 128], BF16, tag="po", name=f"tp{pair}{j}")
                nc.tensor.matmul(
                    tp[:],
                    yn[pair][j].rearrange("p a b -> p (a b)"),
                    idn[:],
                    start=True, stop=True, is_transpose=True,
                )
                nc.gpsimd.tensor_copy(
                    out=yt[:, pair, 128 * j:128 * (j + 1)], in_=tp[:])

        # ---- c_proj split in time: pair-0 partial into SBUF early,
        # pair-1 matmul + merge-add + DMA late ----
        def proj_part0(t, oc):
            if oc == 0:
                pp0[t] = pp0p.tile([128, 1024], BF16, tag="pp0",
                                   name=f"pp0_{t}")
            pp = po.tile([128, TCH], F32, tag="po", name=f"pa{t}{oc}")
            nc.tensor.matmul(
                pp[:], yt[:, 0, 128 * t:128 * (t + 1)],
                wp2[:, 0, TCH * oc:TCH * (oc + 1)],
                start=True, stop=True)
            eng = nc.vector if oc == 1 else nc.gpsimd
            eng.tensor_copy(out=pp0[t][:, TCH * oc:TCH * (oc + 1)],
                            in_=pp[:])

        stg_live = {}

        def proj_final(t, oc):
            if oc == 0:
                stg_live[t] = ostp.tile([128, 1024], BF16, tag="stg",
                                        name=f"stg{t}")
            stg = stg_live[t]
            pp = po.tile([128, TCH], F32, tag="po", name=f"pb{t}{oc}")
            mm2(pp[:], yt[:, :, 128 * t:128 * (t + 1)],
                wp2[:, :, TCH * oc:TCH * (oc + 1)], True, True, YP8)
            eng = nc.vector if oc == 1 else nc.gpsimd
            eng.tensor_copy(out=stg[:, TCH * oc:TCH * (oc + 1)], in_=pp[:])
            if oc == 1:
                nc.sync.dma_start(out=out_d[128 * t:128 * (t + 1), :],
                                  in_=stg[:])
                del stg_live[t]

        # ---- S tile: row-packed K=64 bf16 matmuls, 512-col psum banks ----
        def emit_S(h, half, t):
            pair, h2 = divmod(h, 2)
            pb = 64 * h2
            st = ps.tile([128, 1024], F32, tag="st", name=f"st{h}{half}{t}")
            kb = qk[2 + pair][pb:pb + 64, 128 * t:128 * (t + 1)]
            lo = (128 * t - 1024 * half) if (t // 8) == half else 0
            a = lo
            while a < 1024:
                b = min((a // TCH + 1) * TCH, 1024)
                nc.tensor.matmul(
                    st[:, a:b], kb,
                    qk[pair][pb:pb + 64, 1024 * half + a:1024 * half + b],
                    start=True, stop=True)
                a = b
            return st

        # ---- AV burst for out-tile block jj: slot-paired over adjacent
        # tk tiles, odd tail as a plain matmul ----
        def emit_AV(h, t, jj, pts2):
            avt = po.tile([128, HS + 1], F32, tag="po", name=f"av{h}{t}")
            n = t + 1
            np2 = n // 2
            for s2 in range(np2):
                mm2(avt[:], pts2[s2][:, :, 128 * jj:128 * (jj + 1)],
                    v_all[:, 2 * s2:2 * s2 + 2, h, :],
                    s2 == 0, (s2 == np2 - 1) and (n % 2 == 0), PV8)
            if n % 2:
                nc.tensor.matmul(
                    avt[:],
                    pts2[n // 2][:, 0, 128 * jj:128 * (jj + 1)],
                    v_all[:, n - 1, h, :],
                    start=(np2 == 0), stop=True)
            return avt

        pdt = F8 if PV8 else BF16

        # AV bursts run one step behind exp: the burst emitted at step t
        # is for tile t-1, whose exp finished a full step ago — the PE
        # never parks on a fresh exp's pipeline+semaphore latency. The
        # last burst of a phase is flushed by the next phase's step 0.
        pending_av = []

        def flush_av(force=True):
            if not force and len(pending_av) < 2:
                return
            while pending_av:
                h_, t_, jj_, pts2_ = pending_av.pop(0)
                pair_, h2_ = divmod(h_, 2)
                avt = emit_AV(h_, t_, jj_, pts2_)
                drain_j(pair_, h2_, t_, avt)

        def attn(h, half, sched=None, st0=None, prelude=None,
                 eager_from=None):
            pair, h2 = divmod(h, 2)
            t_end = 8 * (half + 1)
            pts2 = {}
            st = st0 if st0 is not None else emit_S(h, half, 0)
            pre = None
            sched = sched or {}
            for t in range(t_end):
                diag = (t // 8) == half
                lo = (128 * t - 1024 * half) if diag else 0
                s2, par = divmod(t, 2)
                if par == 0:
                    pts2[s2] = ptp.tile([128, 2, 1024], pdt, tag="pt",
                                        name=f"pt{h}{half}{s2}")
                # exp(S/8 - 2): -2 rescales P by e^-2 (cancels in the
                # softmax ratio; guards fp8 range if pt goes fp8)
                nc.scalar.activation(
                    out=pts2[s2][:, par, lo:1024], in_=st[:, lo:1024],
                    func=AF.Exp, scale=0.125, bias=neg2[:],
                )
                if diag:
                    # zero the strict-lower (tk > tq) of the diag block
                    nc.vector.tensor_mul(
                        out=pts2[s2][:, par, lo:lo + 128],
                        in0=pts2[s2][:, par, lo:lo + 128],
                        in1=msk[:],
                    )
                # one filler to cover the S psum-slot wait, then S(t+1)
                # so exp(t+1) is never late, remaining fillers, then the
                # latency-tolerant tp/AV flushes (producers one step old).
                # NB: a filler that reads yt tile j must be scheduled at
                # least one step after tp(j) flushed (write-before-read).
                units = sched.get(t, [])
                if units:
                    units[0]()
                if t + 1 < t_end:
                    st = emit_S(h, half, t + 1)
                elif prelude is not None:
                    pre = prelude()
                for u in units[1:]:
                    u()
                eager = eager_from is not None and t >= eager_from
                flush_tp(force=eager)
                flush_av(force=eager)
                jj = t - 8 * half
                if jj >= 0:
                    pending_av.append((h, t, jj, pts2))
            flush_tp()
            return pre

        # -------- schedule --------
        # Phase order: both heads' half0, then half1, per pair.
        # h = 2*pair + h2. Fillers are spread so every exp-bound phase
        # deficit is covered by mobile PE work whose deps allow it.
        def fl(*units):
            # units: (step, callable) pairs -> per-step schedule dict
            d = {}
            for st_, fn in units:
                d.setdefault(st_, []).append(fn)
            return d

        def P0(t, oc):
            return lambda: proj_part0(t, oc)

        def PF(t, oc):
            return lambda: proj_final(t, oc)

        def QS(m, i):
            return lambda: qk_sub(m, i)

        def VT(t):
            return lambda: v_tile(t)

        qk_upfront()
        for t in range(3):
            v_tile(t)

        e1 = [(t, VT(3 + t)) for t in range(5)]
        s0 = attn(0, 0, sched=fl(*e1),
                  prelude=lambda: emit_S(1, 0, 0))
        s0 = attn(1, 0, st0=s0,
                  sched=fl(*[(t, VT(8 + t)) for t in range(5)]),
                  prelude=lambda: emit_S(0, 1, 0))

        e3 = [(0, VT(13)), (1, VT(14)), (2, VT(15)), (4, QS(1, 0))]
        s0 = attn(0, 1, st0=s0, sched=fl(*e3),
                  prelude=lambda: emit_S(1, 1, 0))
        e4 = [(1, QS(3, 0)), (3, QS(1, 1)), (5, QS(3, 1))]
        s0 = attn(1, 1, st0=s0, sched=fl(*e4),
                  prelude=lambda: emit_S(2, 0, 0))

        e5 = [(6, QS(1, 2))]
        s0 = attn(2, 0, st0=s0, sched=fl(*e5),
                  prelude=lambda: emit_S(3, 0, 0))
        e6 = [(1, QS(1, 3)), (3, QS(3, 2))]
        s0 = attn(3, 0, st0=s0, sched=fl(*e6),
                  prelude=lambda: emit_S(2, 1, 0))

        # (2,1): last q/k sub + pair-1 finals for tiles 0-7 (their yt
        # slot-1 columns completed during (3,0))
        e7 = [(0, QS(3, 3))] + \
            [(1 + 2 * k + oc, PF(k, oc)) for k in range(7)
             for oc in (0, 1)]
        s0 = attn(2, 1, st0=s0, sched=fl(*e7),
                  prelude=lambda: emit_S(3, 1, 0))
        # (3,1): pair-0 partials for tiles 8-15 fill the early steps;
        # finals chase the drains from step 10
        e8 = [(10, PF(7, 0)), (10, PF(7, 1))] + \
            [(11 + k, PF(8 + k, oc)) for k in range(5) for oc in (0, 1)]
        attn(3, 1, st0=s0, sched=fl(*e8), eager_from=8)
        flush_av()
        flush_tp()
        for t in (13, 14, 15):
            proj_final(t, 0)
            proj_final(t, 1)
        ostp.release()
        pp0p.release()
        ynp.release()
        lrp.release()
        ptp.release()
        xp.release()


_PROG = None


def _get_program():
    global _PROG
    if _PROG is None:
        _PROG = build_program()
    return _PROG


def _bf(a):
    return np.ascontiguousarray(np.asarray(a, dtype=ml_dtypes.bfloat16))


def _cast(a, f8):
    dt = E4 if f8 else ml_dtypes.bfloat16
    return np.ascontiguousarray(np.asarray(a, dtype=dt))


def make_in_maps(x, w_attn, b_attn, w_proj, b_proj):
    x = np.asarray(x, dtype=np.float32)
    w_attn = np.asarray(w_attn, dtype=np.float32)
    b_attn = np.asarray(b_attn, dtype=np.float32)
    w_proj = np.asarray(w_proj, dtype=np.float32)
    wq, wk, wv = w_attn[:, 0:C], w_attn[:, C:2 * C], w_attn[:, 2 * C:3 * C]
    bq, bk, bv = b_attn[0:C], b_attn[C:2 * C], b_attn[2 * C:3 * C]
    # inclusive upper-tri (tq >= tk keeps) for the S^T diagonal block
    msk = np.triu(np.ones((128, 128), dtype=np.float32))
    in_maps = []
    for core in range(NCORES):
        b, g = divmod(core, 4)
        cs = slice(256 * g, 256 * (g + 1))
        # x^T -> [128, c-pair s, slot j, t]
        x4 = x[b].T.reshape(4, 2, 128, T).transpose(2, 0, 1, 3)
        # chunks m: [q-pair0, q-pair1, k-pair0, k-pair1]
        wqk_cols = np.concatenate([wq[:, cs], wk[:, cs]], axis=1)
        wqk4 = wqk_cols.reshape(4, 2, 128, 512).transpose(2, 0, 1, 3)
        bqk_ = np.concatenate([bq[cs], bk[cs]]).reshape(4, 128).T.copy()
        wv4 = wv[:, cs].reshape(4, 2, 128, 256).transpose(2, 0, 1, 3)
        wp2 = w_proj[cs, :].reshape(2, 128, 1024).transpose(1, 0, 2)
        in_maps.append({
            "x4": _cast(x4, XW8),
            "wqk": _cast(wqk4, XW8),
            "wv": _cast(wv4, XW8),
            "wp": _cast(wp2, YP8),
            "bqk": np.ascontiguousarray(bqk_, dtype=np.float32),
            "bvb": _bf(np.concatenate([
                np.broadcast_to(bv[cs][None, :], (128, 256)),
                np.ones((128, 64), dtype=np.float32)], axis=1)),
            "msk": _bf(msk),
            "idn": _bf(np.eye(128, dtype=np.float32)),
        })
    return in_maps


def gather_output(results, b_proj):
    b_proj = np.asarray(b_proj, dtype=np.float32)
    out = np.empty((B, T, C), dtype=np.float32)
    for b in range(B):
        acc = results[4 * b]["out"].astype(np.float32)
        for g in range(1, 4):
            acc = acc + results[4 * b + g]["out"].astype(np.float32)
        out[b] = acc + b_proj[None, :]
    return out


def kernel(x, w_attn, b_attn, w_proj, b_proj):
    nc = _get_program()
    in_maps = make_in_maps(x, w_attn, b_attn, w_proj, b_proj)
    res = run_bass_kernel_spmd(nc, in_maps, core_ids=list(range(NCORES)))
    return gather_output(res.results, b_proj)


# revision 45
# speedup vs baseline: 1.0149x; 1.0149x over previous
"""Causal self-attention on 8 trn2 NeuronCores (bf16 datapath, fp8-ready).

Problem: B=2, T=2048, C=1024, 16 heads of 64. Sharding: core = 4*b + g
(b = batch, g = head-group of 4 heads). Each core computes QKV projection
for its 4 heads, causal attention, and a partial c_proj (its 256 rows of
w_proj). Host sums the 4 partials per batch (the "all-reduce") + b_proj.

The PE is the binding engine (~100us of matmul at bf16; Act exp is
~76us; DVE/Pool less), so the schedule exists to keep the PE queue fed:
every deferrable matmul (pair-1 q/k projection, V tiles, c_proj) is
emitted as filler inside the exp-gated attention phases.

Layouts are slot-paired ([p, 2, *]) so each matmul class can switch
between one fp8 DoubleRow instruction and two plain bf16 matmuls via
the knobs below (numerics currently demand bf16 everywhere):

  xt[s]   (128, 2, 2048)  x^T, c-tile pair s (c = 256s + 128j + p)
  wqk[s]  (128, 2, 512)   chunks m: q-pair0, q-pair1, k-pair0, k-pair1
  qk[m]   (128, 2048)     q^T/k^T per head-pair chunk (bf16: S matmul
                          operand base partitions are limited to
                          {0,32,64}, ruling out a 4x32 fp8-DR S layout)
  v_all   (128, 16, 4, 65) V'=[V|1] natural; slot pairs = adjacent tk
  pt2     (128, 2, 1024)  P^T pairs: slot = tk-tile parity
  yt      (128, 2, 2048)  y^T: slot = head pair
  wp2     (128, 2, 1024)  w_proj rows: slot = head pair

Attention: head-serial (pair, half, h2) phases, software-pipelined:
S(t+1) and fillers are emitted before the AV burst that parks on
exp(t); exp applies scale=1/8 (weights are kept unscaled) and bias=-2
(rescales P by e^-2, cancels in the softmax ratio; keeps exp inside
fp8 range if pt ever goes fp8). The diagonal block is masked by a DVE
multiply with the inclusive-upper-tri mask. AV runs flipped (stationary
P^T slice, moving V' -> out [tq, 65]); the denominator lands as column
65, normalization is reciprocal + tensor_scalar on DVE, and a PE
transpose (identity moving operand) rebuilds y^T.

c_proj is split in time: the pair-0 half (yt slot 0) is computed into
SBUF partials during phases (0,1,*) and (1,0,*) where the PE would
otherwise idle; the pair-1 half + merge-add + out DMA ride (1,1,*) as
the pair-1 yt columns complete. Act does exp only; Pool (gpsimd) takes
bias adds, tp->yt copies and half of the partial/merge traffic.
"""

import numpy as np
import ml_dtypes

import concourse.tile as tile
from concourse import bacc, mybir
from concourse.bass_utils import run_bass_kernel_spmd

B, T, C = 2, 2048, 1024
HS = 64
NCORES = 8
NHL = 4            # heads per core
TCH = 512
NT = T // 128      # 16 tk tiles
F32 = mybir.dt.float32
BF16 = mybir.dt.bfloat16
F8 = mybir.dt.float8e4
E4 = ml_dtypes.float8_e4m3

# dtype knobs: fp8+DoubleRow per stage (False = bf16, two plain matmuls)
XW8 = False  # x / wqk / wv: QKV projection
PV8 = True  # P / V: AV matmul
YP8 = False  # y^T / wp: c_proj

DR = mybir.MatmulPerfMode.DoubleRow


def build_program():
    nc = bacc.Bacc("TRN2", target_bir_lowering=False, debug=False)

    xdt = F8 if XW8 else BF16
    vdt = F8 if PV8 else BF16
    ydt = F8 if YP8 else BF16

    x4_d = nc.dram_tensor("x4", [128, 4, 2, T], xdt,
                          kind="ExternalInput").ap()
    wqk_d = nc.dram_tensor("wqk", [128, 4, 2, 512], xdt,
                           kind="ExternalInput").ap()
    wv_d = nc.dram_tensor("wv", [128, 4, 2, 256], xdt,
                          kind="ExternalInput").ap()
    wp_d = nc.dram_tensor("wp", [128, 2, 1024], ydt,
                          kind="ExternalInput").ap()
    bqk_d = nc.dram_tensor("bqk", [128, 4], F32, kind="ExternalInput").ap()
    bvb_d = nc.dram_tensor("bvb", [128, 320], BF16,
                           kind="ExternalInput").ap()
    msk_d = nc.dram_tensor("msk", [128, 128], BF16,
                           kind="ExternalInput").ap()
    idn_d = nc.dram_tensor("idn", [128, 128], BF16,
                           kind="ExternalInput").ap()
    out_d = nc.dram_tensor("out", [T, C], BF16, kind="ExternalOutput").ap()

    with tile.TileContext(nc) as tc:
        _kernel(tc, out_d, x4_d, wqk_d, wv_d, wp_d, bqk_d, bvb_d, msk_d,
                idn_d, xdt, vdt, ydt)
    nc.compile()
    return nc


def _kernel(tc, out_d, x4_d, wqk_d, wv_d, wp_d, bqk_d, bvb_d, msk_d,
            idn_d, xdt, vdt, ydt):
    nc = tc.nc
    AF = mybir.ActivationFunctionType

    def mm2(out, lhsT, rhs, start, stop, f8):
        # slot-paired matmul: lhsT/rhs [p, 2, *]. One DoubleRow fp8
        # instruction, or two plain matmuls over the slots.
        if f8:
            nc.tensor.matmul(out, lhsT, rhs, start=start, stop=stop,
                             perf_mode=DR)
        else:
            nc.tensor.matmul(out, lhsT[:, 0], rhs[:, 0], start=start,
                             stop=False)
            nc.tensor.matmul(out, lhsT[:, 1], rhs[:, 1], start=False,
                             stop=stop)

    with (
        tc.tile_pool(name="persist", bufs=1) as pers,
        tc.tile_pool(name="ps", bufs=2, space="PSUM") as ps,
        tc.tile_pool(name="po", bufs=4, space="PSUM") as po,
    ):
        # zero operands for PE-warmup matmuls; memset first so the ramp
        # warmups start before any DMA lands
        zs = pers.tile([64, 128], BF16, tag="zs")
        nc.vector.memset(zs[:], 0)
        zs2 = pers.tile([64, 512], BF16, tag="zs2")
        nc.vector.memset(zs2[:], 0)
        neg2 = pers.tile([128, 1], F32, tag="neg2")
        nc.vector.memset(neg2[:], -2.0)

        xp = tc.alloc_tile_pool(name="xp", bufs=1)
        xt = []
        for s in range(4):
            t_ = xp.tile([128, 2, T], xdt, tag=f"xt{s}", name=f"xt{s}")
            # two sub-DMAs per c-tile pair so consumption paces the stream
            nc.sync.dma_start(out=t_[:, 0, :], in_=x4_d[:, s, 0])
            nc.sync.dma_start(out=t_[:, 1, :], in_=x4_d[:, s, 1])
            xt.append(t_)
        wqk = []
        for s in range(4):
            w_ = pers.tile([128, 2, 512], xdt, tag=f"wqk{s}",
                           name=f"wqk{s}")
            nc.scalar.dma_start(out=w_, in_=wqk_d[:, s])
            wqk.append(w_)
        bqk = pers.tile([128, 4], F32, tag="bqk")
        nc.scalar.dma_start(out=bqk, in_=bqk_d)
        msk = pers.tile([128, 128], BF16, tag="msk")
        nc.scalar.dma_start(out=msk, in_=msk_d)
        wv = []
        for s in range(4):
            t_ = pers.tile([128, 2, 256], xdt, tag=f"wv{s}", name=f"wv{s}")
            nc.sync.dma_start(out=t_, in_=wv_d[:, s])
            wv.append(t_)
        bvb = pers.tile([128, 320], BF16, tag="bvb")
        nc.sync.dma_start(out=bvb, in_=bvb_d)
        idn = pers.tile([128, 128], BF16, tag="idn")
        nc.scalar.dma_start(out=idn, in_=idn_d)
        wp2 = pers.tile([128, 2, 1024], ydt, tag="wp2")
        nc.scalar.dma_start(out=wp2, in_=wp_d)

        # q^T / k^T chunks: m 0/1 -> q head-pairs, 2/3 -> k head-pairs
        qk = [pers.tile([128, T], BF16, tag=f"qk{m}", name=f"qk{m}")
              for m in range(4)]
        # V' = [V | 1] per (tk-tile, head); ones col from bvb[:, 256:320]
        v_all = pers.tile([128, NT, NHL, HS + 1], vdt, tag="v_all",
                          name="v_all")
        nc.vector.tensor_copy(
            out=v_all[:, :, :, HS],
            in_=bvb[:, 256:320].rearrange("p (a b) -> p a b", a=NT),
        )
        # y^T, slot = head pair
        yt = pers.tile([128, 2, T], ydt, tag="yt", name="yt")

        def warm(pst, n):
            for i in range(n):
                nc.tensor.matmul(
                    pst[0:128, 0:512], zs[:], zs2[:],
                    start=(i == 0), stop=False, skip_group_check=True,
                )

        # ---- QKV projection group (m-chunk, tq i-chunk) ----
        def qk_add(m, i, pg):
            eng = nc.vector if (m + i) % 2 == 0 else nc.gpsimd
            eng.tensor_scalar_add(
                out=qk[m][:, TCH * i:TCH * (i + 1)],
                in0=pg[:],
                scalar1=bqk[:, m:m + 1],
            )

        def qk_sub(m, i):
            pg = po.tile([128, TCH], F32, tag="po", name=f"pq{m}_{i}")
            for s in range(4):
                mm2(pg[:], wqk[s][:, :, 128 * m:128 * (m + 1)],
                    xt[s][:, :, TCH * i:TCH * (i + 1)], s == 0, s == 3,
                    XW8)
            qk_add(m, i, pg)

        def qk_upfront():
            # pair-0 q/k (m=0,2) x all i: 8 concurrent groups (4 po +
            # 2x2 ps halves), s-major so the PE paces the x DMA stream
            pgs = []
            for i in range(2):
                for m in (0, 2):
                    pgs.append((m, i, po.tile([128, TCH], F32, tag="po",
                                              name=f"pg{m}_{i}")))
            wide = [ps.tile([128, 1024], F32, tag="st", name=f"pw{i}")
                    for i in range(2)]
            for i in range(2, 4):
                for m in (0, 2):
                    pgs.append((m, i, wide[i - 2][:, TCH * (m // 2):
                                                  TCH * (m // 2 + 1)]))
            warm(pgs[0][2], 3)
            for s in range(4):
                for m, i, pg in pgs:
                    mm2(pg[:], wqk[s][:, :, 128 * m:128 * (m + 1)],
                        xt[s][:, :, TCH * i:TCH * (i + 1)], s == 0,
                        s == 3, XW8)
            for m, i, pg in pgs:
                qk_add(m, i, pg)

        # ---- V tile t (natural layout, +bias) ----
        def v_tile(t):
            vp = po.tile([128, 256], F32, tag="po", name=f"vp{t}")
            for s in range(4):
                mm2(vp[:], xt[s][:, :, 128 * t:128 * (t + 1)], wv[s][:],
                    s == 0, s == 3, XW8)
            nc.gpsimd.tensor_add(
                out=v_all[:, t, :, 0:HS],
                in0=vp[:].rearrange("p (h d) -> p h d", h=NHL),
                in1=bvb[:, 0:256].rearrange("p (h d) -> p h d", h=NHL),
            )

        # ---- attention-side SBUF pools ----
        ptp = tc.alloc_tile_pool(name="pt", bufs=14)
        lrp = tc.alloc_tile_pool(name="lrec", bufs=8)
        ynp = tc.alloc_tile_pool(name="ynp", bufs=40)
        pp0p = tc.alloc_tile_pool(name="pp0", bufs=16)
        ostp = tc.alloc_tile_pool(name="ost", bufs=4)
        yn = [[None] * NT for _ in range(2)]
        pp0 = [None] * NT
        pending_tp = []

        def drain_j(pair, h2, j, av):
            if h2 == 0:
                yn[pair][j] = ynp.tile([128, 2, HS], BF16, tag="yn",
                                       name=f"yn{pair}{j}")
            lr = lrp.tile([128, 1], F32, tag="lr", name=f"lr{pair}{h2}{j}")
            nc.vector.reciprocal(out=lr[:], in_=av[:, HS:HS + 1])
            nc.vector.tensor_scalar_mul(
                out=yn[pair][j][:, h2, :],
                in0=av[:, 0:HS],
                scalar1=lr[:],
            )
            if h2 == 1:
                pending_tp.append((pair, j))

        def flush_tp(force=True):
            if not force and len(pending_tp) < 2:
                return
            while pending_tp:
                pair, j = pending_tp.pop(0)
                tp = po.tile([128, 128], BF16, tag="po", name=f"tp{pair}{j}")
                nc.tensor.matmul(
                    tp[:],
                    yn[pair][j].rearrange("p a b -> p (a b)"),
                    idn[:],
                    start=True, stop=True, is_transpose=True,
                )
                nc.gpsimd.tensor_copy(
                    out=yt[:, pair, 128 * j:128 * (j + 1)], in_=tp[:])

        # ---- c_proj split in time: pair-0 partial into SBUF early,
        # pair-1 matmul + merge-add + DMA late ----
        def proj_part0(t, oc):
            if oc == 0:
                pp0[t] = pp0p.tile([128, 1024], BF16, tag="pp0",
                                   name=f"pp0_{t}")
            pp = po.tile([128, TCH], F32, tag="po", name=f"pa{t}{oc}")
            nc.tensor.matmul(
                pp[:], yt[:, 0, 128 * t:128 * (t + 1)],
                wp2[:, 0, TCH * oc:TCH * (oc + 1)],
                start=True, stop=True)
            eng = nc.vector if oc == 1 else nc.gpsimd
            eng.tensor_copy(out=pp0[t][:, TCH * oc:TCH * (oc + 1)],
                            in_=pp[:])

        stg_live = {}

        def proj_final(t, oc):
            if oc == 0:
                stg_live[t] = ostp.tile([128, 1024], BF16, tag="stg",
                                        name=f"stg{t}")
            stg = stg_live[t]
            pp = po.tile([128, TCH], F32, tag="po", name=f"pb{t}{oc}")
            mm2(pp[:], yt[:, :, 128 * t:128 * (t + 1)],
                wp2[:, :, TCH * oc:TCH * (oc + 1)], True, True, YP8)
            eng = nc.vector if oc == 1 else nc.gpsimd
            eng.tensor_copy(out=stg[:, TCH * oc:TCH * (oc + 1)], in_=pp[:])
            if oc == 1:
                nc.sync.dma_start(out=out_d[128 * t:128 * (t + 1), :],
                                  in_=stg[:])
                del stg_live[t]

        # ---- S tile: row-packed K=64 bf16 matmuls, 512-col psum banks ----
        def emit_S(h, half, t):
            pair, h2 = divmod(h, 2)
            pb = 64 * h2
            st = ps.tile([128, 1024], F32, tag="st", name=f"st{h}{half}{t}")
            kb = qk[2 + pair][pb:pb + 64, 128 * t:128 * (t + 1)]
            lo = (128 * t - 1024 * half) if (t // 8) == half else 0
            a = lo
            while a < 1024:
                b = min((a // TCH + 1) * TCH, 1024)
                nc.tensor.matmul(
                    st[:, a:b], kb,
                    qk[pair][pb:pb + 64, 1024 * half + a:1024 * half + b],
                    start=True, stop=True)
                a = b
            return st

        # ---- AV burst for out-tile block jj: slot-paired over adjacent
        # tk tiles, odd tail as a plain matmul ----
        def emit_AV(h, t, jj, pts2):
            avt = po.tile([128, HS + 1], F32, tag="po", name=f"av{h}{t}")
            n = t + 1
            np2 = n // 2
            for s2 in range(np2):
                mm2(avt[:], pts2[s2][:, :, 128 * jj:128 * (jj + 1)],
                    v_all[:, 2 * s2:2 * s2 + 2, h, :],
                    s2 == 0, (s2 == np2 - 1) and (n % 2 == 0), PV8)
            if n % 2:
                nc.tensor.matmul(
                    avt[:],
                    pts2[n // 2][:, 0, 128 * jj:128 * (jj + 1)],
                    v_all[:, n - 1, h, :],
                    start=(np2 == 0), stop=True)
            return avt

        pdt = F8 if PV8 else BF16

        # AV bursts run one step behind exp: the burst emitted at step t
        # is for tile t-1, whose exp finished a full step ago — the PE
        # never parks on a fresh exp's pipeline+semaphore latency. The
        # last burst of a phase is flushed by the next phase's step 0.
        pending_av = []

        def flush_av(force=True):
            if not force and len(pending_av) < 2:
                return
            while pending_av:
                h_, t_, jj_, pts2_ = pending_av.pop(0)
                pair_, h2_ = divmod(h_, 2)
                avt = emit_AV(h_, t_, jj_, pts2_)
                drain_j(pair_, h2_, t_, avt)

        def attn(h, half, sched=None, st0=None, prelude=None,
                 eager_from=None):
            pair, h2 = divmod(h, 2)
            t_end = 8 * (half + 1)
            pts2 = {}
            st = st0 if st0 is not None else emit_S(h, half, 0)
            pre = None
            sched = sched or {}
            for t in range(t_end):
                diag = (t // 8) == half
                lo = (128 * t - 1024 * half) if diag else 0
                s2, par = divmod(t, 2)
                if par == 0:
                    pts2[s2] = ptp.tile([128, 2, 1024], pdt, tag="pt",
                                        name=f"pt{h}{half}{s2}")
                # exp(S/8 - 2): -2 rescales P by e^-2 (cancels in the
                # softmax ratio; guards fp8 range if pt goes fp8)
                nc.scalar.activation(
                    out=pts2[s2][:, par, lo:1024], in_=st[:, lo:1024],
                    func=AF.Exp, scale=0.125, bias=neg2[:],
                )
                if diag:
                    # zero the strict-lower (tk > tq) of the diag block
                    nc.vector.tensor_mul(
                        out=pts2[s2][:, par, lo:lo + 128],
                        in0=pts2[s2][:, par, lo:lo + 128],
                        in1=msk[:],
                    )
                # one filler to cover the S psum-slot wait, then S(t+1)
                # so exp(t+1) is never late, remaining fillers, then the
                # latency-tolerant tp/AV flushes (producers one step old).
                # NB: a filler that reads yt tile j must be scheduled at
                # least one step after tp(j) flushed (write-before-read).
                units = sched.get(t, [])
                if units:
                    units[0]()
                if t + 1 < t_end:
                    st = emit_S(h, half, t + 1)
                elif prelude is not None:
                    pre = prelude()
                for u in units[1:]:
                    u()
                eager = eager_from is not None and t >= eager_from
                flush_tp(force=eager)
                flush_av(force=eager)
                jj = t - 8 * half
                if jj >= 0:
                    pending_av.append((h, t, jj, pts2))
            flush_tp()
            return pre

        # -------- schedule --------
        # Phase order: both heads' half0, then half1, per pair.
        # h = 2*pair + h2. Fillers are spread so every exp-bound phase
        # deficit is covered by mobile PE work whose deps allow it.
        def fl(*units):
            # units: (step, callable) pairs -> per-step schedule dict
            d = {}
            for st_, fn in units:
                d.setdefault(st_, []).append(fn)
            return d

        def P0(t, oc):
            return lambda: proj_part0(t, oc)

        def PF(t, oc):
            return lambda: proj_final(t, oc)

        def QS(m, i):
            return lambda: qk_sub(m, i)

        def VT(t):
            return lambda: v_tile(t)

        qk_upfront()
        for t in range(3):
            v_tile(t)

        e1 = [(t, VT(3 + t)) for t in range(5)]
        s0 = attn(0, 0, sched=fl(*e1),
                  prelude=lambda: emit_S(1, 0, 0))
        s0 = attn(1, 0, st0=s0,
                  sched=fl(*[(t, VT(8 + t)) for t in range(5)]),
                  prelude=lambda: emit_S(0, 1, 0))

        e3 = [(0, VT(13)), (1, VT(14)), (2, VT(15)), (4, QS(1, 0))]
        s0 = attn(0, 1, st0=s0, sched=fl(*e3),
                  prelude=lambda: emit_S(1, 1, 0))
        e4 = [(1, QS(3, 0)), (3, QS(1, 1)), (5, QS(3, 1))]
        s0 = attn(1, 1, st0=s0, sched=fl(*e4),
                  prelude=lambda: emit_S(2, 0, 0))

        e5 = [(6, QS(1, 2))]
        s0 = attn(2, 0, st0=s0, sched=fl(*e5),
                  prelude=lambda: emit_S(3, 0, 0))
        e6 = [(1, QS(1, 3)), (3, QS(3, 2))]
        s0 = attn(3, 0, st0=s0, sched=fl(*e6),
                  prelude=lambda: emit_S(2, 1, 0))

        # (2,1): last q/k sub + pair-1 finals for tiles 0-7 (their yt
        # slot-1 columns completed during (3,0))
        e7 = [(0, QS(3, 3))] + \
            [(1 + 2 * k + oc, PF(k, oc)) for k in range(7)
             for oc in (0, 1)]
        s0 = attn(2, 1, st0=s0, sched=fl(*e7),
                  prelude=lambda: emit_S(3, 1, 0))
        # (3,1): pair-0 partials for tiles 8-15 fill the early steps;
        # finals chase the drains from step 10
        e8 = [(10, PF(7, 0)), (10, PF(7, 1))] + \
            [(11 + k, PF(8 + k, oc)) for k in range(5) for oc in (0, 1)]
        attn(3, 1, st0=s0, sched=fl(*e8), eager_from=8)
        flush_av()
        flush_tp()
        for t in (13, 14, 15):
            proj_final(t, 0)
            proj_final(t, 1)
        ostp.release()
        pp0p.release()
        ynp.release()
        lrp.release()
        ptp.release()
        xp.release()


_PROG = None


def _get_program():
    global _PROG
    if _PROG is None:
        _PROG = build_program()
    return _PROG


def _bf(a):
    return np.ascontiguousarray(np.asarray(a, dtype=ml_dtypes.bfloat16))


def _cast(a, f8):
    dt = E4 if f8 else ml_dtypes.bfloat16
    return np.ascontiguousarray(np.asarray(a, dtype=dt))


def make_in_maps(x, w_attn, b_attn, w_proj, b_proj):
    x = np.asarray(x, dtype=np.float32)
    w_attn = np.asarray(w_attn, dtype=np.float32)
    b_attn = np.asarray(b_attn, dtype=np.float32)
    w_proj = np.asarray(w_proj, dtype=np.float32)
    wq, wk, wv = w_attn[:, 0:C], w_attn[:, C:2 * C], w_attn[:, 2 * C:3 * C]
    bq, bk, bv = b_attn[0:C], b_attn[C:2 * C], b_attn[2 * C:3 * C]
    # inclusive upper-tri (tq >= tk keeps) for the S^T diagonal block
    msk = np.triu(np.ones((128, 128), dtype=np.float32))
    in_maps = []
    for core in range(NCORES):
        b, g = divmod(core, 4)
        cs = slice(256 * g, 256 * (g + 1))
        # x^T -> [128, c-pair s, slot j, t]
        x4 = x[b].T.reshape(4, 2, 128, T).transpose(2, 0, 1, 3)
        # chunks m: [q-pair0, q-pair1, k-pair0, k-pair1]
        wqk_cols = np.concatenate([wq[:, cs], wk[:, cs]], axis=1)
        wqk4 = wqk_cols.reshape(4, 2, 128, 512).transpose(2, 0, 1, 3)
        bqk_ = np.concatenate([bq[cs], bk[cs]]).reshape(4, 128).T.copy()
        wv4 = wv[:, cs].reshape(4, 2, 128, 256).transpose(2, 0, 1, 3)
        wp2 = w_proj[cs, :].reshape(2, 128, 1024).transpose(1, 0, 2)
        in_maps.append({
            "x4": _cast(x4, XW8),
            "wqk": _cast(wqk4, XW8),
            "wv": _cast(wv4, XW8),
            "wp": _cast(wp2, YP8),
            "bqk": np.ascontiguousarray(bqk_, dtype=np.float32),
            "bvb": _bf(np.concatenate([
                np.broadcast_to(bv[cs][None, :], (128, 256)),
                np.ones((128, 64), dtype=np.float32)], axis=1)),
            "msk": _bf(msk),
            "idn": _bf(np.eye(128, dtype=np.float32)),
        })
    return in_maps


def gather_output(results, b_proj):
    b_proj = np.asarray(b_proj, dtype=np.float32)
    out = np.empty((B, T, C), dtype=np.float32)
    for b in range(B):
        acc = results[4 * b]["out"].astype(np.float32)
        for g in range(1, 4):
            acc = acc + results[4 * b + g]["out"].astype(np.float32)
        out[b] = acc + b_proj[None, :]
    return out


def kernel(x, w_attn, b_attn, w_proj, b_proj):
    nc = _get_program()
    in_maps = make_in_maps(x, w_attn, b_attn, w_proj, b_proj)
    res = run_bass_kernel_spmd(nc, in_maps, core_ids=list(range(NCORES)))
    return gather_output(res.results, b_proj)
